# revision 1
# baseline (speedup 1.0000x reference)
"""Trainium2 Bass kernel for nn_Model_11149735100878 (biaffine dependency parser).

Architecture: embeddings -> 3-layer BiLSTM (H=400) -> fencepost -> 2 MLPs -> biaffine.
Sharding: data-parallel over batch, 4 batches per core, 8 cores, identical SPMD
program (no collectives; per-core inputs differ only in the token/batch data).

Self-contained: hardcodes all shapes; imports concourse from the axon site path.
"""
import sys, os

for _p in ("/root/.axon_site/_ro/trn_rl_repo",):
    if _p not in sys.path:
        sys.path.insert(0, _p)

import numpy as np

# ---------------- model dims (hardcoded from the problem spec) ----------------
B, S, V, NT, NE, H, NL, NM = 32, 128, 32000, 64, 100, 400, 128, 100
T = S                 # 128 time steps
Bc = 4                # batches per core
NCORES = 8
NTOK = T * Bc         # 512 (t,b) tokens per core
X = S - 1             # 127 fencepost positions

GORDER = [0, 1, 3, 2]  # torch gate rows (i,f,g,o) -> our order (i,f,o,g)

# build flags (overridable for partial testing)
N_LAYERS = 3
DO_HEAD = True        # MLP + biaffine + output
DEBUG_SEQ = False     # expose seq_dram as an output


# ================================ host prep =================================

def _valid_u(kt, v):
    """u = 32*kt + (v&31); valid if < 100. Returns (u, valid)."""
    u = 32 * kt + (v & 31)
    return u, u < 100


def prep_shared(word_emb, tag_emb, Wih0, Wih1, Wih2, Whh, b,
                mlp_l_W, mlp_l_b, mlp_r_W, mlp_r_b, biaffine_W):
    """Preprocess weights into device layouts (shared by all cores)."""
    f32 = np.float32
    out = {}
    out["word_emb"] = np.ascontiguousarray(word_emb, f32)
    out["tag_emb"] = np.ascontiguousarray(tag_emb, f32)

    v_idx = np.arange(128)
    jh_v = v_idx >> 5          # [128]
    jll_v = v_idx & 31

    # ---- Whh -> wm_all [6 (l*2+d), 128 v, 4 kt, 4 jh_o, 400 (g,u)] ----
    # col c (within jh_o block) = g*100 + u ; value = W4[g, jh_o*100+u, j_in(kt,v)]
    wm_all = np.zeros((6, 128, 4, 4, 400), f32)
    # j_out array for cols: [4 jh_o, 400] -> true row in W4[g]: jh_o*100 + u
    for l in range(3):
        for d in range(2):
            W4 = Whh[l, d].reshape(4, 400, 400)[GORDER]  # [g, j_out, j_in]
            for kt in range(4):
                u_in = 32 * kt + jll_v                    # [128]
                valid = u_in < 100
                j_in = jh_v * 100 + np.minimum(u_in, 99)  # clamp, mask later
                # block[v, jh_o, g, u_o] = W4[g, jh_o*100+u_o, j_in[v]]
                blk = W4[:, :, j_in]                      # [g, 1600?, 128] -> [4, 400*?]:
                # W4[:, :, j_in]: [4, 400, 128]; reshape j_out as (jh_o, u_o):
                blk = blk.reshape(4, 4, 100, 128)         # [g, jh_o, u_o, v]
                blk = blk.transpose(3, 1, 0, 2)           # [v, jh_o, g, u_o]
                blk = blk.reshape(128, 4, 400) * valid[:, None, None]
                wm_all[l * 2 + d, :, kt] = blk
    out["wm_all"] = wm_all.reshape(6, 128, 6400)

    # ---- Wih -> wih blocks [l][d][kt, 128 v, 1600 (jh_o, g, u)] ----
    def wih_block(Wihld, x_col, valid):
        # Wihld [1600, IN]; returns [128, 1600] for this kt
        Wg = Wihld.reshape(4, 400, -1)[GORDER]            # [g, j_out, IN]
        blk = Wg[:, :, x_col]                             # [g, 400, 128]
        blk = blk.reshape(4, 4, 100, 128).transpose(3, 1, 0, 2).reshape(128, 1600)
        return (blk * valid[:, None]).astype(f32)

    wih0 = np.zeros((2, 2, 128, 1600), f32)
    for d in range(2):
        # kt0: word feats (v<100), kt1: tag feats
        val = v_idx < 100
        xc0 = np.minimum(v_idx, 99)
        wih0[d, 0] = wih_block(Wih0[d], xc0, val)
        wih0[d, 1] = wih_block(Wih0[d], 100 + xc0, val)
    out["wih0"] = wih0

    wih12 = np.zeros((2, 2, 8, 128, 1600), f32)
    for li, Wih in enumerate((Wih1, Wih2)):
        for d in range(2):
            for kt in range(8):
                din, jlh = kt // 4, kt % 4
                u_in = 32 * jlh + jll_v
                valid = u_in < 100
                xc = din * 400 + jh_v * 100 + np.minimum(u_in, 99)
                wih12[li, d, kt] = wih_block(Wih[d], xc, valid)
    out["wih12"] = wih12

    # ---- biases -> bias_blk [6, 128, 1600]: row 0 = b reordered ----
    bias_blk = np.zeros((6, 128, 1600), f32)
    for l in range(3):
        for d in range(2):
            bg = b[l, d].reshape(4, 400)[GORDER]          # [g, j]
            # cols (jh_o, g, u): bg[g, jh_o*100+u]
            bb = bg.reshape(4, 4, 100).transpose(1, 0, 2).reshape(1600)
            bias_blk[l * 2 + d, 0] = bb
    out["bias_blk"] = bias_blk

    # ---- MLP weights [2 (l/r), 8 kt, 128 v, 100] ----
    wmlp = np.zeros((2, 8, 128, 100), f32)
    for i, W in enumerate((mlp_l_W, mlp_r_W)):
        for kt in range(8):
            din, jlh = kt // 4, kt % 4
            u_in = 32 * jlh + jll_v
            valid = u_in < 100
            xc = din * 400 + jh_v * 100 + np.minimum(u_in, 99)
            wmlp[i, kt] = (W[:, xc].T * valid[:, None]).astype(f32)
    out["wmlp"] = wmlp

    mlp_bias = np.zeros((2, 128, 1), f32)
    mlp_bias[0, :100, 0] = mlp_l_b
    mlp_bias[1, :100, 0] = mlp_r_b
    out["mlp_bias"] = mlp_bias

    # ---- biaffine [101 i, 128*101 (o,j)] ----
    out["biaf"] = np.ascontiguousarray(
        biaffine_W.transpose(1, 0, 2).reshape(101, NL * 101), f32)
    return out


def prep_core_tokens(words, tags, core):
    """Per-core token indices in (t, b) order, int32 [512, 1]."""
    bs = slice(4 * core, 4 * core + 4)
    w = np.ascontiguousarray(words[bs].T.reshape(NTOK, 1)).astype(np.int32)
    t = np.ascontiguousarray(tags[bs].T.reshape(NTOK, 1)).astype(np.int32)
    return w, t


# ================================ device build ==============================

def build(n_layers=N_LAYERS, do_head=DO_HEAD, debug_seq=DEBUG_SEQ):
    from concourse import bass, bacc, mybir, tile
    from concourse.masks import make_identity
    F32 = mybir.dt.float32
    I32 = mybir.dt.int32
    AF = mybir.ActivationFunctionType

    nc = bacc.Bacc(None, target_bir_lowering=False)

    # ---- I/O ----
    words_tb = nc.dram_tensor("words_tb", [NTOK, 1], I32, kind="ExternalInput")
    tags_tb = nc.dram_tensor("tags_tb", [NTOK, 1], I32, kind="ExternalInput")
    word_emb = nc.dram_tensor("word_emb", [V, NE], F32, kind="ExternalInput")
    tag_emb = nc.dram_tensor("tag_emb", [NT, NE], F32, kind="ExternalInput")
    wm_all = nc.dram_tensor("wm_all", [6, 128, 6400], F32, kind="ExternalInput")
    wih0_d = nc.dram_tensor("wih0", [2, 2, 128, 1600], F32, kind="ExternalInput")
    wih12_d = nc.dram_tensor("wih12", [2, 2, 8, 128, 1600], F32, kind="ExternalInput")
    bias_blk = nc.dram_tensor("bias_blk", [6, 128, 1600], F32, kind="ExternalInput")
    wmlp_d = nc.dram_tensor("wmlp", [2, 8, 128, 100], F32, kind="ExternalInput")
    mlp_bias_d = nc.dram_tensor("mlp_bias", [2, 128, 1], F32, kind="ExternalInput")
    biaf_d = nc.dram_tensor("biaf", [101, NL * 101], F32, kind="ExternalInput")

    if do_head:
        out_d = nc.dram_tensor("out", [Bc, X, X, NL], F32, kind="ExternalOutput")
    if debug_seq:
        dbg_seq = nc.dram_tensor("dbg_seq", [n_layers, 2, 128, T * 16], F32,
                                 kind="ExternalOutput")

    with tile.TileContext(nc) as tc:
        const = tc.alloc_tile_pool(name="const", bufs=1)
        dr = tc.alloc_tile_pool(name="dr", bufs=1, space="DRAM")
        sb = tc.alloc_tile_pool(name="sb", bufs=1)
        sbr = tc.alloc_tile_pool(name="sbr", bufs=4)     # rotating small tiles
        ps = tc.alloc_tile_pool(name="ps", bufs=1, space="PSUM")
        psr = tc.alloc_tile_pool(name="psr", bufs=2, space="PSUM")

        # ---------- constants ----------
        id128 = const.tile([128, 128], F32, tag="id128", name="id128")
        make_identity(nc, id128[:])
        ones_st = const.tile([128, 128], F32, tag="ones_st", name="ones_st")   # row 0 = 1
        nc.vector.memset(ones_st[:], 0.0)
        nc.vector.memset(ones_st[0:1, :], 1.0)
        alpha_t = const.tile([128, 1], F32, tag="alpha", name="alpha")
        nc.vector.memset(alpha_t[:], 0.1)

        # ---------- DRAM scratch ----------
        seq_dram = [[dr.tile([128, T * 16], F32, tag=f"seq{l}{d}", name=f"seq{l}{d}")
                     for d in range(2)] for l in range(n_layers)]
        proj_dram = [[dr.tile([NTOK, 1600], F32, tag=f"proj{l}{d}", name=f"proj{l}{d}")
                      for d in range(2)] for l in range(n_layers)]

        # ---------- embeddings -> x0T (feature-major) ----------
        x0T = [sb.tile([128, NTOK], F32, tag=f"x0T{k}", name=f"x0T{k}") for k in range(2)]
        for k in range(2):
            nc.vector.memset(x0T[k][:], 0.0)
        for pt in range(4):
            tok_sl = slice(128 * pt, 128 * pt + 128)
            for k, (table, idx_d, nrow) in enumerate(
                    ((word_emb, words_tb, V), (tag_emb, tags_tb, NT))):
                idx_t = sbr.tile([128, 1], I32, tag="gidx", name="gidx")
                nc.sync.dma_start(out=idx_t[:], in_=idx_d[tok_sl, :])
                gat = sbr.tile([128, NE], F32, tag="gat", name="gat")
                nc.gpsimd.indirect_dma_start(
                    out=gat[:], out_offset=None, in_=table[:, :],
                    in_offset=bass.IndirectOffsetOnAxis(ap=idx_t[:, 0:1], axis=0))
                tp = psr.tile([NE, 128], F32, tag="embT", name="embT")
                nc.tensor.transpose(out=tp[:], in_=gat[:], identity=id128[:])
                nc.scalar.copy(out=x0T[k][0:NE, tok_sl], in_=tp[:])

        # ---------- LSTM weights tiles (reloaded per layer) ----------
        wm_t = [sb.tile([128, 6400], F32, tag=f"wm{d}", name=f"wm{d}") for d in range(2)]
        bias_t = [sb.tile([128, 1600], F32, tag=f"bias{d}", name=f"bias{d}") for d in range(2)]

        # persistent state tiles per dir
        c_t = [sb.tile([128, 100], F32, tag=f"c{d}", name=f"c{d}") for d in range(2)]
        h_t = [sb.tile([128, 128], F32, tag=f"h{d}", name=f"h{d}") for d in range(2)]
        hT_t = [sb.tile([128, 128], F32, tag=f"hT{d}", name=f"hT{d}") for d in range(2)]
        sg_t = [sb.tile([128, 300], F32, tag=f"sg{d}", name=f"sg{d}") for d in range(2)]
        tg_t = [sb.tile([128, 100], F32, tag=f"tg{d}", name=f"tg{d}") for d in range(2)]
        t1_t = [sb.tile([128, 100], F32, tag=f"t1{d}", name=f"t1{d}") for d in range(2)]
        t2_t = [sb.tile([128, 100], F32, tag=f"t2{d}", name=f"t2{d}") for d in range(2)]
        tc_t = [sb.tile([128, 100], F32, tag=f"tc{d}", name=f"tc{d}") for d in range(2)]
        gates = [ps.tile([128, 400], F32, tag=f"gates{d}", name=f"gates{d}") for d in range(2)]
        for d in range(2):
            nc.vector.memset(gates[d][:], 0.0)
            nc.vector.memset(h_t[d][:], 0.0)

        proj_ps = ps.tile([128, 2048], F32, tag="proj_ps", name="proj_ps")
        xt_tiles = [sb.tile([128, 128], F32, tag=f"xt{i}", name=f"xt{i}") for i in range(8)]
        wih_t = [sb.tile([128, 1600], F32, tag=f"wih{i}", name=f"wih{i}") for i in range(8)]

        for l in range(n_layers):
            nkt = 2 if l == 0 else 8
            # -- load layer weights --
            for d in range(2):
                nc.sync.dma_start(out=wm_t[d][:], in_=wm_all[l * 2 + d, :, :])
                nc.sync.dma_start(out=bias_t[d][:], in_=bias_blk[l * 2 + d, :, :])

            # -- x stationary tiles for proj --
            # l==0: x0T word/tag tiles used directly (slice per ptile)
            if l > 0:
                sq = [seq_dram[l - 1][din][:].rearrange(
                    "p (t q b) -> p t q b", t=T, q=4, b=4) for din in range(2)]

            # -- proj big matmul, both dirs --
            for d in range(2):
                for kt in range(nkt):
                    if l == 0:
                        nc.sync.dma_start(out=wih_t[kt][:], in_=wih0_d[d, kt, :, :])
                    else:
                        nc.sync.dma_start(out=wih_t[kt][:], in_=wih12_d[l - 1, d, kt, :, :])
                for pt in range(4):
                    tok_sl = slice(128 * pt, 128 * pt + 128)
                    # load x stationary tiles for this ptile
                    if l > 0:
                        for kt in range(8):
                            din, jlh = kt // 4, kt % 4
                            nc.sync.dma_start(
                                out=xt_tiles[kt][:],
                                in_=sq[din][:, 32 * pt:32 * pt + 32, jlh, :])
                    pp4 = proj_ps[:].rearrange("p (a c) -> p a c", a=4, c=512)
                    for jh in range(4):
                        # bias first (start), then x contributions
                        nc.tensor.matmul(
                            out=pp4[:, jh, 0:400], lhsT=ones_st[:],
                            rhs=bias_t[d][:, jh * 400:(jh + 1) * 400],
                            start=True, stop=False, tile_position=(0, 0))
                        for kt in range(nkt):
                            if l == 0:
                                lhsT = x0T[kt][:, tok_sl]
                            else:
                                lhsT = xt_tiles[kt][:]
                            nc.tensor.matmul(
                                out=pp4[:, jh, 0:400], lhsT=lhsT,
                                rhs=wih_t[kt][:, jh * 400:(jh + 1) * 400],
                                start=False, stop=(kt == nkt - 1),
                                tile_position=(0, 0))
                    # copy PSUM -> SBUF -> proj_dram
                    pj_sb = sbr.tile([128, 1600], F32, tag="pj_sb", name="pj_sb")
                    nc.scalar.copy(out=pj_sb[:].rearrange("p (a c) -> p a c", a=4, c=400),
                                   in_=pp4[:, :, 0:400])
                    nc.sync.dma_start(out=proj_dram[l][d][tok_sl, :], in_=pj_sb[:])

            # -- recurrence --
            for d in range(2):
                nc.vector.memset(c_t[d][:], 0.0)
                nc.vector.memset(hT_t[d][:], 0.0)

            wm4s = [wm_t[d][:].rearrange("p (k a c) -> p k a c", k=4, a=4, c=400)
                    for d in range(2)]
            for t in range(T):
                tbs = (t, T - 1 - t)
                # matmuls (both dirs)
                for d in range(2):
                    tb = tbs[d]
                    g = gates[d]
                    pj = sbr.tile([4, 1600], F32, tag=f"pjs{d}", name=f"pjs{d}")
                    nc.sync.dma_start(out=pj[:], in_=proj_dram[l][d][4 * tb:4 * tb + 4, :])
                    pj4 = pj[:].rearrange("p (a c) -> p a c", a=4, c=400)
                    for jh in range(4):
                        nc.tensor.matmul(
                            out=g[32 * jh:32 * jh + 4, :],
                            lhsT=id128[0:4, 0:4], rhs=pj4[:, jh, :],
                            start=True, stop=False, tile_position=(0, 32 * jh))
                        for kt in range(4):
                            nc.tensor.matmul(
                                out=g[32 * jh:32 * jh + 4, :],
                                lhsT=hT_t[d][:, 32 * kt:32 * kt + 4],
                                rhs=wm4s[d][:, kt, jh, :],
                                start=False, stop=(kt == 3),
                                tile_position=(0, 32 * jh))
                # gate nonlinearities (ACT stream: sig_f, tanh_f, sig_b, tanh_b)
                for d in range(2):
                    nc.scalar.activation(out=sg_t[d][:], in_=gates[d][:, 0:300], func=AF.Sigmoid)
                    nc.scalar.activation(out=tg_t[d][:], in_=gates[d][:, 300:400], func=AF.Tanh)
                # cell state updates (DVE) + tanh(c) (ACT, queued after both sigmoids)
                for d in range(2):
                    nc.vector.tensor_mul(out=t1_t[d][:], in0=c_t[d][:], in1=sg_t[d][:, 100:200])
                    nc.vector.tensor_mul(out=t2_t[d][:], in0=sg_t[d][:, 0:100], in1=tg_t[d][:])
                    nc.vector.tensor_add(out=c_t[d][:], in0=t1_t[d][:], in1=t2_t[d][:])
                    nc.scalar.activation(out=tc_t[d][:], in_=c_t[d][:], func=AF.Tanh)
                # h, transpose, seq store
                for d in range(2):
                    tb = tbs[d]
                    nc.vector.tensor_mul(out=h_t[d][:, 0:100], in0=sg_t[d][:, 200:300], in1=tc_t[d][:])
                    nc.vector.transpose(out=hT_t[d][:], in_=h_t[d][:])
                    hT4 = hT_t[d][:].rearrange("p (a c) -> p a c", a=4, c=32)
                    nc.sync.dma_start(
                        out=seq_dram[l][d][:, 16 * tb:16 * tb + 16],
                        in_=hT4[:, :, 0:4])

        if debug_seq:
            for l in range(n_layers):
                for d in range(2):
                    nc.sync.dma_start(out=dbg_seq[l, d, :, :], in_=seq_dram[l][d][:])

        for p in (psr, ps, sbr, sb):
            p.release()

        if do_head:
            _emit_head(nc, tc, seq_dram, wmlp_d, mlp_bias_d,
                       biaf_d, alpha_t, out_d, mybir)

        for p in (dr, const):
            p.release()

    nc.finalize()
    return nc


def _emit_head(nc, tc, seq_dram, wmlp_d, mlp_bias_d, biaf_d,
               alpha_t, out_d, mybir):
    """Fencepost -> MLPs -> biaffine -> output."""
    F32 = mybir.dt.float32
    AF = mybir.ActivationFunctionType
    NX = X * Bc  # 508
    sb = tc.alloc_tile_pool(name="hsb", bufs=1)
    sbr = tc.alloc_tile_pool(name="hsbr", bufs=4)
    psr = tc.alloc_tile_pool(name="hpsr", bufs=2, space="PSUM")
    xfp = tc.alloc_tile_pool(name="xfp", bufs=1)

    sq = [seq_dram[-1][din][:].rearrange("p (t q b) -> p t q b", t=T, q=4, b=4)
          for din in range(2)]
    # fencepost x tiles [128, 508] per kt=(din, jlh): fwd t in [0,127), bwd t in [1,128)
    xf_tiles = []
    for kt in range(8):
        din, jlh = kt // 4, kt % 4
        t0 = 0 if din == 0 else 1
        xt = xfp.tile([128, NX], F32, tag=f"xf{kt}", name=f"xf{kt}")
        nc.sync.dma_start(out=xt[:], in_=sq[din][:, t0:t0 + X, jlh, :])
        xf_tiles.append(xt)

    # MLPs -> l1T / r1T [128 (101 used), 512]
    wtiles = [sb.tile([128, 100], F32, tag=f"wmlp{i}", name=f"wmlp{i}") for i in range(8)]
    lr1T = []
    for mi in range(2):
        for kt in range(8):
            nc.sync.dma_start(out=wtiles[kt][:], in_=wmlp_d[mi, kt, :, :])
        bt = sbr.tile([128, 1], F32, tag="mlpb_t", name="mlpb_t")
        nc.sync.dma_start(out=bt[:], in_=mlp_bias_d[mi, :, :])
        mp = psr.tile([100, NX], F32, tag="mlp_ps", name="mlp_ps")
        for kt in range(8):
            nc.tensor.matmul(out=mp[:], lhsT=wtiles[kt][:, :], rhs=xf_tiles[kt][:],
                             start=(kt == 0), stop=(kt == 7), tile_position=(0, 0))
        ot = sb.tile([128, 512], F32, tag=f"lr1T{mi}", name=f"lr1T{mi}")
        nc.vector.memset(ot[:], 0.0)
        nc.vector.memset(ot[96:128, :], 1.0)  # row 100 -> 1.0; 96-99 overwritten below
        nc.scalar.activation(out=ot[0:100, 0:NX], in_=mp[:], func=AF.Prelu,
                             bias=bt[0:100, 0:1], alpha=alpha_t[0:100, 0:1])
        lr1T.append(ot)
    l1T, r1T = lr1T

    # biaffine weights (xf tiles no longer needed; free their pool first)
    xfp.release()
    bsb = tc.alloc_tile_pool(name="bsb", bufs=1)
    biaf_t = bsb.tile([101, NL * 101], F32, tag="biaf_t", name="biaf_t")
    nc.sync.dma_start(out=biaf_t[:], in_=biaf_d[:, :])
    biaf3 = biaf_t[:].rearrange("p (o j) -> p o j", o=NL, j=101)

    # tmp for a batch PAIR: [101 j, (x, 2, o)]
    tmp2 = bsb.tile([101, X * 2 * NL], F32, tag="tmp2", name="tmp2")
    tmp4 = tmp2[:].rearrange("p (x c o) -> p x c o", x=X, c=2, o=NL)

    l1v = l1T[0:101, 0:NX].rearrange("p (x c) -> p x c", x=X, c=4)
    r1v = r1T[0:101, 0:NX].rearrange("p (x c) -> p x c", x=X, c=4)
    XC = 4  # x-chunk for step 2

    for bp in range(2):
        l1_pair = l1v[:, :, 2 * bp:2 * bp + 2]          # [101, 127, 2] -> N=254
        # step 1: tmp[j, x, bb, o] = sum_i W[o,i,j] * l1[b,x,i]
        for o in range(NL):
            ps1 = psr.tile([101, 2 * X], F32, tag="ps1", name="ps1")
            nc.tensor.matmul(out=ps1[:], lhsT=biaf3[:, o, :], rhs=l1_pair,
                             start=True, stop=True, tile_position=(0, 0))
            pv = ps1[:].rearrange("p (x c) -> p x c", x=X, c=2)
            if o % 2 == 0:
                nc.scalar.copy(out=tmp4[:, :, :, o], in_=pv)
            else:
                nc.vector.tensor_copy(out=tmp4[:, :, :, o], in_=pv)
        # step 2: s[b, x, y, o] = sum_j r1[b, y, j] * tmp[j, x, bb, o]
        for bb in range(2):
            b = 2 * bp + bb
            r1_b = r1v[:, :, b]
            for x0 in range(0, X, XC):
                nx = min(XC, X - x0)
                ps2 = psr.tile([X, XC * NL], F32, tag="ps2", name="ps2")
                nc.tensor.matmul(out=ps2[:, 0:nx * NL],
                                 lhsT=r1_b, rhs=tmp4[:, x0:x0 + nx, bb, :],
                                 start=True, stop=True, tile_position=(0, 0))
                so = sbr.tile([X, XC * NL], F32, tag="so", name="so", bufs=2)
                if x0 % 8 == 0:
                    nc.scalar.copy(out=so[:, 0:nx * NL], in_=ps2[:, 0:nx * NL])
                else:
                    nc.vector.tensor_copy(out=so[:, 0:nx * NL], in_=ps2[:, 0:nx * NL])
                nc.sync.dma_start(
                    out=out_d[b, x0:x0 + nx, :, :].rearrange("x y o -> y x o"),
                    in_=so[:, 0:nx * NL].rearrange("p (x o) -> p x o", x=nx, o=NL))

    for p in (bsb, psr, sbr, sb):
        p.release()


# ================================ entry point ===============================

def kernel(**inputs):
    from concourse.bass_utils import run_bass_kernel_spmd

    np_in = {k: np.asarray(v) for k, v in inputs.items()}
    shared = prep_shared(
        np_in["word_emb"], np_in["tag_emb"], np_in["Wih0"], np_in["Wih1"],
        np_in["Wih2"], np_in["Whh"], np_in["b"], np_in["mlp_l_W"],
        np_in["mlp_l_b"], np_in["mlp_r_W"], np_in["mlp_r_b"],
        np_in["biaffine_W"])

    nc = build()
    in_maps = []
    for c in range(NCORES):
        w, t = prep_core_tokens(np_in["words"], np_in["tags"], c)
        m = dict(shared)
        m["words_tb"] = w
        m["tags_tb"] = t
        in_maps.append(m)

    res = run_bass_kernel_spmd(nc, in_maps, core_ids=list(range(NCORES)))
    out = np.empty((B, X, X, NL), np.float32)
    for c in range(NCORES):
        out[4 * c:4 * c + 4] = res.results[c]["out"]
    return out



# revision 40
# speedup vs baseline: 11921.4918x; 11921.4918x over previous
"""Trainium2 Bass kernel for nn_Model_11149735100878 (biaffine dependency parser).

Architecture: embeddings -> 3-layer BiLSTM (H=400) -> fencepost -> 2 MLPs -> biaffine.
Sharding: data-parallel over batch, 4 batches per core, 8 cores, identical SPMD
program (no collectives; per-core inputs differ only in the token/batch data).

Self-contained: hardcodes all shapes; imports concourse from the axon site path.
"""
import sys, os

for _p in ("/root/.axon_site/_ro/trn_rl_repo",):
    if _p not in sys.path:
        sys.path.insert(0, _p)

import numpy as np

# ---------------- model dims (hardcoded from the problem spec) ----------------
B, S, V, NT, NE, H, NL, NM = 32, 128, 32000, 64, 100, 400, 128, 100
T = S                 # 128 time steps
Bc = 4                # batches per core
NCORES = 8
NTOK = T * Bc         # 512 (t,b) tokens per core
X = S - 1             # 127 fencepost positions

GORDER = [0, 1, 3, 2]  # torch gate rows (i,f,g,o) -> our order (i,f,o,g)

# build flags (overridable for partial testing)
N_LAYERS = 3
DO_HEAD = True        # MLP + biaffine + output
DEBUG_SEQ = False     # expose seq_dram as an output


# ================================ host prep =================================

def _valid_u(kt, v):
    """u = 32*kt + (v&31); valid if < 100. Returns (u, valid)."""
    u = 32 * kt + (v & 31)
    return u, u < 100


def prep_shared(word_emb, tag_emb, Wih0, Wih1, Wih2, Whh, b,
                mlp_l_W, mlp_l_b, mlp_r_W, mlp_r_b, biaffine_W):
    """Preprocess weights into device layouts (shared by all cores)."""
    f32 = np.float32
    out = {}
    out["word_emb"] = np.ascontiguousarray(word_emb, f32)
    out["tag_emb"] = np.ascontiguousarray(tag_emb, f32)

    v_idx = np.arange(128)
    jh_v = v_idx >> 5          # [128]
    jll_v = v_idx & 31

    # ---- Whh -> wm_all [6 (l*2+d), 128 v, 4 kt, 4 jh_o, 400 (g,u)] ----
    # col c (within jh_o block) = g*100 + u ; value = W4[g, jh_o*100+u, j_in(kt,v)]
    wm_all = np.zeros((6, 128, 4, 4, 400), f32)
    # j_out array for cols: [4 jh_o, 400] -> true row in W4[g]: jh_o*100 + u
    for l in range(3):
        for d in range(2):
            W4 = Whh[l, d].reshape(4, 400, 400)[GORDER]  # [g, j_out, j_in]
            for kt in range(4):
                u_in = 32 * kt + jll_v                    # [128]
                valid = u_in < 100
                j_in = jh_v * 100 + np.minimum(u_in, 99)  # clamp, mask later
                # block[v, jh_o, g, u_o] = W4[g, jh_o*100+u_o, j_in[v]]
                blk = W4[:, :, j_in]                      # [g, 1600?, 128] -> [4, 400*?]:
                # W4[:, :, j_in]: [4, 400, 128]; reshape j_out as (jh_o, u_o):
                blk = blk.reshape(4, 4, 100, 128)         # [g, jh_o, u_o, v]
                blk = blk.transpose(3, 1, 0, 2)           # [v, jh_o, g, u_o]
                blk = blk.reshape(128, 4, 400) * valid[:, None, None]
                wm_all[l * 2 + d, :, kt] = blk
    out["wm_all"] = wm_all.reshape(6, 128, 6400)

    # ---- Wih -> wih blocks [l][d][kt, 128 v, 1600 (jh_o, g, u)] ----
    def wih_block(Wihld, x_col, valid):
        # Wihld [1600, IN]; returns [128, 1600] for this kt
        Wg = Wihld.reshape(4, 400, -1)[GORDER]            # [g, j_out, IN]
        blk = Wg[:, :, x_col]                             # [g, 400, 128]
        blk = blk.reshape(4, 4, 100, 128).transpose(3, 1, 0, 2).reshape(128, 1600)
        return (blk * valid[:, None]).astype(f32)

    wih0 = np.zeros((2, 2, 128, 1600), f32)
    for d in range(2):
        # kt0: word feats (v<100), kt1: tag feats
        val = v_idx < 100
        xc0 = np.minimum(v_idx, 99)
        wih0[d, 0] = wih_block(Wih0[d], xc0, val)
        wih0[d, 1] = wih_block(Wih0[d], 100 + xc0, val)
    out["wih0"] = wih0

    wih12 = np.zeros((2, 2, 8, 128, 1600), f32)
    for li, Wih in enumerate((Wih1, Wih2)):
        for d in range(2):
            for kt in range(8):
                din, jlh = kt // 4, kt % 4
                u_in = 32 * jlh + jll_v
                valid = u_in < 100
                xc = din * 400 + jh_v * 100 + np.minimum(u_in, 99)
                wih12[li, d, kt] = wih_block(Wih[d], xc, valid)
    out["wih12"] = wih12

    # ---- biases -> bias_blk [6, 128, 1600]: row 0 = b reordered ----
    bias_blk = np.zeros((6, 128, 1600), f32)
    for l in range(3):
        for d in range(2):
            bg = b[l, d].reshape(4, 400)[GORDER]          # [g, j]
            # cols (jh_o, g, u): bg[g, jh_o*100+u]
            bb = bg.reshape(4, 4, 100).transpose(1, 0, 2).reshape(1600)
            bias_blk[l * 2 + d, 0] = bb
    out["bias_blk"] = bias_blk

    # ---- MLP weights [2 (l/r), 8 kt, 128 v, 100] ----
    wmlp = np.zeros((2, 8, 128, 100), f32)
    for i, W in enumerate((mlp_l_W, mlp_r_W)):
        for kt in range(8):
            din, jlh = kt // 4, kt % 4
            u_in = 32 * jlh + jll_v
            valid = u_in < 100
            xc = din * 400 + jh_v * 100 + np.minimum(u_in, 99)
            wmlp[i, kt] = (W[:, xc].T * valid[:, None]).astype(f32)
    out["wmlp"] = wmlp

    mlp_bias = np.zeros((2, 128, 1), f32)
    mlp_bias[0, :100, 0] = mlp_l_b
    mlp_bias[1, :100, 0] = mlp_r_b
    out["mlp_bias"] = mlp_bias

    # ---- biaffine [101 i, 128*101 (o,j)] ----
    out["biaf"] = np.ascontiguousarray(
        biaffine_W.transpose(1, 0, 2).reshape(101, NL * 101), f32)

    # matmul operands are bf16 on device
    import ml_dtypes
    bf16 = ml_dtypes.bfloat16
    for k in ("wm_all", "wih0", "wih12", "bias_blk", "wmlp", "biaf"):
        out[k] = out[k].astype(bf16)
    return out


def prep_core_tokens(words, tags, core):
    """Per-core token indices in (t, b) order, int32 [512, 1]."""
    bs = slice(4 * core, 4 * core + 4)
    w = np.ascontiguousarray(words[bs].T.reshape(NTOK, 1)).astype(np.int32)
    t = np.ascontiguousarray(tags[bs].T.reshape(NTOK, 1)).astype(np.int32)
    return w, t


# ================================ device build ==============================

def build(n_layers=N_LAYERS, do_head=DO_HEAD, debug_seq=DEBUG_SEQ):
    from concourse import bass, bacc, mybir, tile
    from concourse.masks import make_identity
    F32 = mybir.dt.float32
    BF16 = mybir.dt.bfloat16
    I32 = mybir.dt.int32
    AF = mybir.ActivationFunctionType

    nc = bacc.Bacc(None, target_bir_lowering=False)

    def mmr(out, lhsT, rhs, **kw):
        # bf16 matmul: 1 cycle/row (vs 4 for fp32); fp32 accumulate in PSUM
        nc.tensor.matmul(out=out, lhsT=lhsT, rhs=rhs, **kw)

    # ---- I/O ----
    words_tb = nc.dram_tensor("words_tb", [NTOK, 1], I32, kind="ExternalInput")
    tags_tb = nc.dram_tensor("tags_tb", [NTOK, 1], I32, kind="ExternalInput")
    word_emb = nc.dram_tensor("word_emb", [V, NE], F32, kind="ExternalInput")
    tag_emb = nc.dram_tensor("tag_emb", [NT, NE], F32, kind="ExternalInput")
    wm_all = nc.dram_tensor("wm_all", [6, 128, 6400], BF16, kind="ExternalInput")
    wih0_d = nc.dram_tensor("wih0", [2, 2, 128, 1600], BF16, kind="ExternalInput")
    wih12_d = nc.dram_tensor("wih12", [2, 2, 8, 128, 1600], BF16, kind="ExternalInput")
    bias_blk = nc.dram_tensor("bias_blk", [6, 128, 1600], BF16, kind="ExternalInput")
    wmlp_d = nc.dram_tensor("wmlp", [2, 8, 128, 100], BF16, kind="ExternalInput")
    mlp_bias_d = nc.dram_tensor("mlp_bias", [2, 128, 1], F32, kind="ExternalInput")
    biaf_d = nc.dram_tensor("biaf", [101, NL * 101], BF16, kind="ExternalInput")

    if do_head:
        # [b, y, x, o] layout (host swaps x/y back); bf16 halves output DMA
        out_d = nc.dram_tensor("out", [Bc, X, X, NL], BF16, kind="ExternalOutput")
    if debug_seq:
        dbg_seq = nc.dram_tensor("dbg_seq", [n_layers, 2, 128, T * 16], F32,
                                 kind="ExternalOutput")

    with tile.TileContext(nc) as tc:
        const = tc.alloc_tile_pool(name="const", bufs=1)
        dr = tc.alloc_tile_pool(name="dr", bufs=1, space="DRAM")
        seqp = tc.alloc_tile_pool(name="seqp", bufs=1)
        sb = tc.alloc_tile_pool(name="sb", bufs=1)
        sbr = tc.alloc_tile_pool(name="sbr", bufs=4)     # rotating small tiles
        ps = tc.alloc_tile_pool(name="ps", bufs=1, space="PSUM")
        psr = tc.alloc_tile_pool(name="psr", bufs=2, space="PSUM")

        # ---------- constants ----------
        id128 = const.tile([128, 128], F32, tag="id128", name="id128")
        make_identity(nc, id128[:])
        idr = const.tile([128, 128], BF16, tag="idr", name="idr")
        make_identity(nc, idr[:])
        ones_st = const.tile([128, 128], BF16, tag="ones_st", name="ones_st")   # row 0 = 1
        nc.vector.memset(ones_st[:], 0.0)
        nc.vector.memset(ones_st[0:1, :], 1.0)
        alpha_t = const.tile([128, 1], F32, tag="alpha", name="alpha")
        nc.vector.memset(alpha_t[:], 0.1)

        # ---------- DRAM scratch ----------
        proj_dram = [[dr.tile([NTOK, 1600], BF16, tag=f"proj{l}{d}", name=f"proj{l}{d}")
                      for d in range(2)] for l in range(n_layers)]

        # ---------- hidden sequences stay in SBUF (4KB/partition each) ----------
        # col layout (q, t, b) so per-(q) t-slices are contiguous: proj lhsT
        # and head rhs slices need a single flat free dim
        seq_slots = [[seqp.tile([128, T * 16], BF16, tag=f"seq{s}{d}", name=f"seq{s}{d}")
                      for d in range(2)] for s in range(2)]
        seq_sb = [seq_slots[l % 2] for l in range(n_layers)]

        # ---------- head weights: preload now, consumed after the layers ----
        wtiles = [[seqp.tile([128, 100], BF16, tag=f"wmlp{mi}{i}", name=f"wmlp{mi}{i}")
                   for i in range(8)] for mi in range(2)]
        bt_t = [seqp.tile([128, 1], F32, tag=f"mlpb{mi}", name=f"mlpb{mi}")
                for mi in range(2)]
        biaf_t = seqp.tile([101, NL * 101], BF16, tag="biaf_t", name="biaf_t")
        for mi in range(2):
            for kt in range(8):
                nc.gpsimd.dma_start(out=wtiles[mi][kt][:], in_=wmlp_d[mi, kt, :, :])
            nc.gpsimd.dma_start(out=bt_t[mi][:], in_=mlp_bias_d[mi, :, :])
        nc.gpsimd.dma_start(out=biaf_t[:], in_=biaf_d[:, :])
        head_w = (wtiles, bt_t, biaf_t)

        # ---------- embeddings -> x0T (feature-major) ----------
        x0T = [sb.tile([128, NTOK], BF16, tag=f"x0T{k}", name=f"x0T{k}") for k in range(2)]
        for k in range(2):
            nc.vector.memset(x0T[k][:], 0.0)
        for pt in range(4):
            tok_sl = slice(128 * pt, 128 * pt + 128)
            for k, (table, idx_d, nrow) in enumerate(
                    ((word_emb, words_tb, V), (tag_emb, tags_tb, NT))):
                idx_t = sbr.tile([128, 1], I32, tag="gidx", name="gidx")
                nc.sync.dma_start(out=idx_t[:], in_=idx_d[tok_sl, :])
                gat = sbr.tile([128, NE], F32, tag="gat", name="gat")
                nc.gpsimd.indirect_dma_start(
                    out=gat[:], out_offset=None, in_=table[:, :],
                    in_offset=bass.IndirectOffsetOnAxis(ap=idx_t[:, 0:1], axis=0))
                tp = psr.tile([NE, 128], F32, tag="embT", name="embT")
                nc.tensor.transpose(out=tp[:], in_=gat[:], identity=id128[:])
                nc.scalar.copy(out=x0T[k][0:NE, tok_sl], in_=tp[:])

        # ---------- LSTM weights tiles (reloaded per layer) ----------
        wm_t = [sb.tile([128, 6400], BF16, tag=f"wm{d}", name=f"wm{d}") for d in range(2)]
        bias_t = [sb.tile([128, 1600], BF16, tag=f"bias{d}", name=f"bias{d}") for d in range(2)]
        # kt=3 weight tiles, parity-rotated: rows (v&31)<4 hold real weights,
        # rows 4..7 receive proj(t) by per-step DMA (dead weight rows hijacked;
        # matching indicator lives in h cols 100:104, see below)
        wm3_t = [[sb.tile([128, 1600], BF16, tag=f"wm3{d}{p}", name=f"wm3{d}{p}")
                  for p in range(2)] for d in range(2)]

        # persistent state tiles per dir
        c_t = [sb.tile([128, 100], F32, tag=f"c{d}", name=f"c{d}") for d in range(2)]
        h_t = [sb.tile([128, 128], BF16, tag=f"h{d}", name=f"h{d}") for d in range(2)]
        hT_t = [sb.tile([128, 128], BF16, tag=f"hT{d}", name=f"hT{d}") for d in range(2)]
        sg_t = [sb.tile([128, 300], F32, tag=f"sg{d}", name=f"sg{d}") for d in range(2)]
        tg_t = [sb.tile([128, 100], F32, tag=f"tg{d}", name=f"tg{d}") for d in range(2)]
        t1_t = [sb.tile([128, 100], F32, tag=f"t1{d}", name=f"t1{d}") for d in range(2)]
        t2_t = [sb.tile([128, 100], F32, tag=f"t2{d}", name=f"t2{d}") for d in range(2)]
        tc_t = [sb.tile([128, 100], F32, tag=f"tc{d}", name=f"tc{d}") for d in range(2)]
        gates = [ps.tile([128, 400], F32, tag=f"gates{d}", name=f"gates{d}") for d in range(2)]
        for d in range(2):
            nc.vector.memset(gates[d][:], 0.0)
            nc.vector.memset(h_t[d][:], 0.0)
            # proj indicator: hT rows 4..7 cols 96:100 become I4 via the
            # block transpose of h[0:4, 100:104] (cols >= 104 stay zero)
            nc.vector.tensor_copy(out=h_t[d][0:4, 100:104], in_=idr[0:4, 0:4])


        wih_t = [sb.tile([128, 1600], BF16, tag=f"wih{i}", name=f"wih{i}") for i in range(8)]

        for l in range(n_layers):
            nkt = 2 if l == 0 else 8
            # -- load layer weights --
            for d in range(2):
                nc.gpsimd.dma_start(out=wm_t[d][:], in_=wm_all[l * 2 + d, :, :])
                nc.gpsimd.dma_start(out=bias_t[d][:], in_=bias_blk[l * 2 + d, :, :])
                for p in range(2):
                    nc.gpsimd.dma_start(out=wm3_t[d][p][:],
                                        in_=wm_all[l * 2 + d, :, 4800:6400])

            # -- x stationary APs for proj (straight out of SBUF seq) --
            if l > 0:
                sq = [seq_sb[l - 1][din][:].rearrange(
                    "p (q t b) -> p q t b", q=4, t=T, b=4) for din in range(2)]

            # -- proj big matmul, both dirs --
            for d in range(2):
                for kt in range(nkt):
                    if l == 0:
                        nc.gpsimd.dma_start(out=wih_t[kt][:], in_=wih0_d[d, kt, :, :])
                    else:
                        nc.gpsimd.dma_start(out=wih_t[kt][:],
                                            in_=wih12_d[l - 1, d, kt, :, :])
                for pt in range(4):
                    tok_sl = slice(128 * pt, 128 * pt + 128)
                    pj_sb = sbr.tile([128, 1600], BF16, tag="pj_sb", name="pj_sb")
                    for jhp in range(2):
                        ppt = ps.tile([128, 1024], F32, tag="projps",
                                      name="projps", bufs=2)
                        pp2 = ppt[:].rearrange("p (a c) -> p a c", a=2, c=512)
                        for jj in range(2):
                            jh = 2 * jhp + jj
                            # bias first (start), then x contributions
                            mmr(out=pp2[:, jj, 0:400], lhsT=ones_st[:],
                                rhs=bias_t[d][:, jh * 400:(jh + 1) * 400],
                                start=True, stop=False, tile_position=(0, 0))
                            for kt in range(nkt):
                                if l == 0:
                                    lhsT = x0T[kt][:, tok_sl]
                                else:
                                    din, jlh = kt // 4, kt % 4
                                    lhsT = sq[din][:, jlh, 32 * pt:32 * pt + 32, :]
                                mmr(out=pp2[:, jj, 0:400], lhsT=lhsT,
                                    rhs=wih_t[kt][:, jh * 400:(jh + 1) * 400],
                                    start=False, stop=(kt == nkt - 1),
                                    tile_position=(0, 0))
                        eng = nc.scalar if jhp == 0 else nc.vector
                        dst = pj_sb[:].rearrange("p (a c) -> p a c", a=4, c=400)
                        if jhp == 0:
                            nc.scalar.copy(out=dst[:, 0:2, :], in_=pp2[:, :, 0:400])
                        else:
                            nc.vector.tensor_copy(out=dst[:, 2:4, :], in_=pp2[:, :, 0:400])
                    nc.sync.dma_start(out=proj_dram[l][d][tok_sl, :], in_=pj_sb[:])

            # -- recurrence --
            for d in range(2):
                nc.vector.memset(c_t[d][:], 0.0)
                nc.vector.memset(hT_t[d][:], 0.0)
                # step-0 proj indicator in hT rows 4..8 (later steps get it
                # from the transpose of h[0:4, 100:104]); DMA because engine
                # writes cannot start at partition 4
                nc.sync.dma_start(out=hT_t[d][4:8, 96:100], in_=idr[0:4, 0:4])
                # prefetch proj for step 0 into wm3 parity tile rows 4..8
                tb0 = 0 if d == 0 else T - 1
                nc.sync.dma_start(out=wm3_t[d][tb0 % 2][4:8, :],
                                  in_=proj_dram[l][d][4 * tb0:4 * tb0 + 4, :])

            wm4s = [wm_t[d][:].rearrange("p (k a c) -> p k a c", k=4, a=4, c=400)
                    for d in range(2)]
            wm3v = [[wm3_t[d][p][:].rearrange("p (a c) -> p a c", a=4, c=400)
                     for p in range(2)] for d in range(2)]
            for t in range(T):
                tbs = (t, T - 1 - t)
                nxt = (t + 1, T - 2 - t)
                # matmuls (both dirs); proj rides in via wm3 rows 4..8
                for d in range(2):
                    tb = tbs[d]
                    g = gates[d]
                    if t + 1 < T:
                        nb = nxt[d]
                        nc.sync.dma_start(out=wm3_t[d][nb % 2][4:8, :],
                                          in_=proj_dram[l][d][4 * nb:4 * nb + 4, :])
                    for jh in range(4):
                        for kt in range(3):
                            mmr(out=g[32 * jh:32 * jh + 4, :],
                                lhsT=hT_t[d][:, 32 * kt:32 * kt + 4],
                                rhs=wm4s[d][:, kt, jh, :],
                                start=(kt == 0), stop=False,
                                tile_position=(0, 32 * jh))
                        mmr(out=g[32 * jh:32 * jh + 4, :],
                            lhsT=hT_t[d][:, 96:100],
                            rhs=wm3v[d][tb % 2][:, jh, :],
                            start=False, stop=True,
                            tile_position=(0, 32 * jh))
                # gate nonlinearities (ACT stream: sig_f, tanh_f, sig_b, tanh_b)
                for d in range(2):
                    nc.scalar.activation(out=sg_t[d][:], in_=gates[d][:, 0:300], func=AF.Sigmoid)
                    nc.scalar.activation(out=tg_t[d][:], in_=gates[d][:, 300:400], func=AF.Tanh)
                # cell state updates (DVE) + tanh(c) (ACT, queued after both sigmoids)
                for d in range(2):
                    nc.vector.tensor_mul(out=t1_t[d][:], in0=c_t[d][:], in1=sg_t[d][:, 100:200])
                    nc.vector.tensor_mul(out=t2_t[d][:], in0=sg_t[d][:, 0:100], in1=tg_t[d][:])
                    nc.vector.tensor_add(out=c_t[d][:], in0=t1_t[d][:], in1=t2_t[d][:])
                    nc.scalar.activation(out=tc_t[d][:], in_=c_t[d][:], func=AF.Tanh)
                # h, transpose, seq store (Pool copy into SBUF seq)
                for d in range(2):
                    tb = tbs[d]
                    nc.vector.tensor_mul(out=h_t[d][:, 0:100], in0=sg_t[d][:, 200:300], in1=tc_t[d][:])
                    nc.vector.transpose(out=hT_t[d][:], in_=h_t[d][:])
                    hT4 = hT_t[d][:].rearrange("p (a c) -> p a c", a=4, c=32)
                    nc.gpsimd.tensor_copy(
                        out=seq_sb[l][d][:].rearrange(
                            "p (q t b) -> p q t b", q=4, t=T, b=4)[:, :, tb, :],
                        in_=hT4[:, :, 0:4])

        if debug_seq:
            for l in range(n_layers):
                for d in range(2):
                    nc.sync.dma_start(out=dbg_seq[l, d, :, :], in_=seq_sb[l][d][:])

        for p in (psr, ps, sbr, sb):
            p.release()

        if do_head:
            _emit_head(nc, tc, seq_sb, head_w, alpha_t, out_d, mybir, mmr)
        seqp.release()

        for p in (dr, const):
            p.release()

    nc.finalize()
    return nc


def _emit_head(nc, tc, seq_dram, head_w, alpha_t, out_d, mybir, mmr):
    """Fencepost -> MLPs -> biaffine -> output.

    Biaffine runs in float32r (1 cyc/row needs out free >= 256):
      step 1: x padded to 128 -> N = 128*2 = 256 per o; o grouped x4 in PSUM
      step 2: x-chunks of 4 with full o -> N = 512
    Output written as [b, y, x, o] in bf16; host swaps x/y and upcasts.
    """
    F32 = mybir.dt.float32
    BF16 = mybir.dt.bfloat16
    AF = mybir.ActivationFunctionType
    NX = X * Bc  # 508
    XP = 128     # x padded (col 127 is zero garbage, never stored)
    sb = tc.alloc_tile_pool(name="hsb", bufs=1)
    sbr = tc.alloc_tile_pool(name="hsbr", bufs=4)
    psr = tc.alloc_tile_pool(name="hpsr", bufs=2, space="PSUM")

    sq = [seq_dram[-1][din][:].rearrange("p (q t b) -> p q t b", q=4, t=T, b=4)
          for din in range(2)]
    # fencepost x slices [128, 508] per kt=(din, jlh): contiguous in SBUF seq
    # (fwd t in [0,127), bwd t in [1,128)) -- used directly as matmul rhs
    xf_tiles = []
    for kt in range(8):
        din, jlh = kt // 4, kt % 4
        t0 = 0 if din == 0 else 1
        xf_tiles.append(sq[din][:, jlh, t0:t0 + X, :])

    # MLPs -> l1T / r1T [128 (101 used), 512]; weights preloaded at start
    wtiles, bt_t, biaf_t = head_w
    lr1T = []
    for mi in range(2):
        bt = bt_t[mi]
        mp = psr.tile([100, NX], F32, tag="mlp_ps", name="mlp_ps")
        for kt in range(8):
            mmr(out=mp[:], lhsT=wtiles[mi][kt][:, :], rhs=xf_tiles[kt],
                start=(kt == 0), stop=(kt == 7), tile_position=(0, 0))
        ot = sb.tile([128, 512], BF16, tag=f"lr1T{mi}", name=f"lr1T{mi}")
        nc.vector.memset(ot[:], 0.0)
        nc.vector.memset(ot[96:128, :], 1.0)  # row 100 -> 1.0; 96-99 overwritten below
        nc.scalar.activation(out=ot[0:100, 0:NX], in_=mp[:], func=AF.Prelu,
                             bias=bt[0:100, 0:1], alpha=alpha_t[0:100, 0:1])
        lr1T.append(ot)
    l1T, r1T = lr1T

    # biaffine weights were preloaded
    bsb = tc.alloc_tile_pool(name="bsb", bufs=1)
    biaf3 = biaf_t[:].rearrange("p (o j) -> p o j", o=NL, j=101)

    # tmp for ALL 4 batches: [101 j, (x_pad, 4, o)] bf16 = 128KB/partition
    tmp2 = bsb.tile([101, XP * 4 * NL], BF16, tag="tmp2", name="tmp2")
    tmp4 = tmp2[:].rearrange("p (x c o) -> p x c o", x=XP, c=4, o=NL)

    # x padded to 128: col 127 is zeros (l1T memset), harmless
    l1v = l1T[0:101, 0:512]                              # [101, 512] -> N=512
    r1v = r1T[0:101, 0:NX].rearrange("p (x c) -> p x c", x=X, c=4)
    XC = 4   # x-chunk for step 2 -> N = 512
    OG = 2   # o's per PSUM group in step 1 (1 bank each, 2 banks per group)

    # step 1: tmp[j, x, b, o] = sum_i W[o,i,j] * l1[b,x,i]  (all 4 batches)
    for og in range(0, NL, OG):
        ps1 = psr.tile([101, OG * 512], F32, tag="ps1", name="ps1")
        p1v = ps1[:].rearrange("p (o n) -> p o n", o=OG, n=512)
        for oi in range(OG):
            mmr(out=p1v[:, oi, :], lhsT=biaf3[:, og + oi, :], rhs=l1v,
                start=True, stop=True, tile_position=(0, 0))
        src = ps1[:].rearrange("p (o x c) -> p x c o", o=OG, x=XP, c=4)
        if (og // OG) % 2 == 0:
            nc.scalar.copy(out=tmp4[:, :, :, og:og + OG], in_=src)
        else:
            nc.vector.tensor_copy(out=tmp4[:, :, :, og:og + OG], in_=src)

    # step 2: s[b, x, y, o] = sum_j r1[b, y, j] * tmp[j, x, b, o]
    # Group 8 x-chunks (32 x's) per output DMA -> 8KB descriptors.
    # SWDGE (gpsimd) spreads packets over all 16 DMA engines but pays
    # ~50ns/descriptor generation on Pool; the HW qSP/qACT lane is one
    # ring (22.5 GB/s) with free generation -- give it ~1/8 of DMAs.
    XG = 2 * XC  # 8 x's per group: 2KB descriptors, many DMAs in flight
    dma_i = 0
    for b in range(4):
        r1_b = r1v[:, :, b]
        for x0 in range(0, X, XG):
            nx_tot = min(XG, X - x0)
            so = sbr.tile([X, XG * NL], BF16, tag="so", name="so", bufs=12)
            for sub in range(0, XG, XC):
                xh = x0 + sub
                if xh >= X:
                    continue
                nxh = min(XC, X - xh)
                ps2 = psr.tile([X, XC * NL], F32, tag="ps2", name="ps2")
                mmr(out=ps2[:], lhsT=r1_b, rhs=tmp4[:, xh:xh + XC, b, :],
                    start=True, stop=True, tile_position=(0, 0))
                csl = slice(sub * NL, sub * NL + nxh * NL)
                psl = slice(0, nxh * NL)
                if (sub // XC) % 2 == 0:
                    nc.scalar.copy(out=so[:, csl], in_=ps2[:, psl])
                else:
                    nc.vector.tensor_copy(out=so[:, csl], in_=ps2[:, psl])
            eng = (nc.sync if dma_i % 16 == 3 else
                   nc.scalar if dma_i % 16 == 11 else nc.gpsimd)
            dma_i += 1
            eng.dma_start(
                out=out_d[b, :, x0:x0 + nx_tot, :],
                in_=so[:, 0:nx_tot * NL].rearrange("p (x o) -> p x o",
                                                   x=nx_tot, o=NL))

    for p in (bsb, psr, sbr, sb):
        p.release()


# ================================ entry point ===============================

def run(np_in, trace=False, tmpdir=None):
    from concourse.bass_utils import run_bass_kernel_spmd

    shared = prep_shared(
        np_in["word_emb"], np_in["tag_emb"], np_in["Wih0"], np_in["Wih1"],
        np_in["Wih2"], np_in["Whh"], np_in["b"], np_in["mlp_l_W"],
        np_in["mlp_l_b"], np_in["mlp_r_W"], np_in["mlp_r_b"],
        np_in["biaffine_W"])

    nc = build()
    in_maps = []
    for c in range(NCORES):
        w, t = prep_core_tokens(np_in["words"], np_in["tags"], c)
        m = dict(shared)
        m["words_tb"] = w
        m["tags_tb"] = t
        in_maps.append(m)

    res = run_bass_kernel_spmd(nc, in_maps, core_ids=list(range(NCORES)),
                               trace=trace, tmpdir=tmpdir)
    out = np.empty((B, X, X, NL), np.float32)
    for c in range(NCORES):
        # device output is [b, y, x, o] bf16 -> upcast + swap to [b, x, y, o]
        oc = np.asarray(res.results[c]["out"]).astype(np.float32)
        out[4 * c:4 * c + 4] = oc.swapaxes(1, 2)
    return out, res


def kernel(**inputs):
    np_in = {k: np.asarray(v) for k, v in inputs.items()}
    out, _ = run(np_in)
    return out



# revision 44
# speedup vs baseline: 12207.9915x; 1.0240x over previous
"""Trainium2 Bass kernel for nn_Model_11149735100878 (biaffine dependency parser).

Architecture: embeddings -> 3-layer BiLSTM (H=400) -> fencepost -> 2 MLPs -> biaffine.
Sharding: data-parallel over batch, 4 batches per core, 8 cores, identical SPMD
program (no collectives; per-core inputs differ only in the token/batch data).

Self-contained: hardcodes all shapes; imports concourse from the axon site path.
"""
import sys, os

for _p in ("/root/.axon_site/_ro/trn_rl_repo",):
    if _p not in sys.path:
        sys.path.insert(0, _p)

import numpy as np

# ---------------- model dims (hardcoded from the problem spec) ----------------
B, S, V, NT, NE, H, NL, NM = 32, 128, 32000, 64, 100, 400, 128, 100
T = S                 # 128 time steps
Bc = 4                # batches per core
NCORES = 8
NTOK = T * Bc         # 512 (t,b) tokens per core
X = S - 1             # 127 fencepost positions

GORDER = [0, 1, 3, 2]  # torch gate rows (i,f,g,o) -> our order (i,f,o,g)

# build flags (overridable for partial testing)
N_LAYERS = 3
DO_HEAD = True        # MLP + biaffine + output
DEBUG_SEQ = False     # expose seq_dram as an output


# ================================ host prep =================================

def _valid_u(kt, v):
    """u = 32*kt + (v&31); valid if < 100. Returns (u, valid)."""
    u = 32 * kt + (v & 31)
    return u, u < 100


def prep_shared(word_emb, tag_emb, Wih0, Wih1, Wih2, Whh, b,
                mlp_l_W, mlp_l_b, mlp_r_W, mlp_r_b, biaffine_W):
    """Preprocess weights into device layouts (shared by all cores)."""
    f32 = np.float32
    out = {}
    out["word_emb"] = np.ascontiguousarray(word_emb, f32)
    out["tag_emb"] = np.ascontiguousarray(tag_emb, f32)

    v_idx = np.arange(128)
    jh_v = v_idx >> 5          # [128]
    jll_v = v_idx & 31

    # ---- Whh -> wm_all [6 (l*2+d), 128 v, 4 kt, 4 jh_o, 400 (g,u)] ----
    # col c (within jh_o block) = g*100 + u ; value = W4[g, jh_o*100+u, j_in(kt,v)]
    wm_all = np.zeros((6, 128, 4, 4, 400), f32)
    # j_out array for cols: [4 jh_o, 400] -> true row in W4[g]: jh_o*100 + u
    for l in range(3):
        for d in range(2):
            W4 = Whh[l, d].reshape(4, 400, 400)[GORDER]  # [g, j_out, j_in]
            for kt in range(4):
                u_in = 32 * kt + jll_v                    # [128]
                valid = u_in < 100
                j_in = jh_v * 100 + np.minimum(u_in, 99)  # clamp, mask later
                # block[v, jh_o, g, u_o] = W4[g, jh_o*100+u_o, j_in[v]]
                blk = W4[:, :, j_in]                      # [g, 1600?, 128] -> [4, 400*?]:
                # W4[:, :, j_in]: [4, 400, 128]; reshape j_out as (jh_o, u_o):
                blk = blk.reshape(4, 4, 100, 128)         # [g, jh_o, u_o, v]
                blk = blk.transpose(3, 1, 0, 2)           # [v, jh_o, g, u_o]
                blk = blk.reshape(128, 4, 400) * valid[:, None, None]
                wm_all[l * 2 + d, :, kt] = blk
    out["wm_all"] = wm_all.reshape(6, 128, 6400)

    # ---- Wih -> wih blocks [l][d][kt, 128 v, 1600 (jh_o, g, u)] ----
    def wih_block(Wihld, x_col, valid):
        # Wihld [1600, IN]; returns [128, 1600] for this kt
        Wg = Wihld.reshape(4, 400, -1)[GORDER]            # [g, j_out, IN]
        blk = Wg[:, :, x_col]                             # [g, 400, 128]
        blk = blk.reshape(4, 4, 100, 128).transpose(3, 1, 0, 2).reshape(128, 1600)
        return (blk * valid[:, None]).astype(f32)

    wih0 = np.zeros((2, 2, 128, 1600), f32)
    for d in range(2):
        # kt0: word feats (v<100), kt1: tag feats
        val = v_idx < 100
        xc0 = np.minimum(v_idx, 99)
        wih0[d, 0] = wih_block(Wih0[d], xc0, val)
        wih0[d, 1] = wih_block(Wih0[d], 100 + xc0, val)
    out["wih0"] = wih0

    wih12 = np.zeros((2, 2, 8, 128, 1600), f32)
    for li, Wih in enumerate((Wih1, Wih2)):
        for d in range(2):
            for kt in range(8):
                din, jlh = kt // 4, kt % 4
                u_in = 32 * jlh + jll_v
                valid = u_in < 100
                xc = din * 400 + jh_v * 100 + np.minimum(u_in, 99)
                wih12[li, d, kt] = wih_block(Wih[d], xc, valid)
    out["wih12"] = wih12

    # ---- biases -> bias_blk [6, 128, 1600]: row 0 = b reordered ----
    bias_blk = np.zeros((6, 128, 1600), f32)
    for l in range(3):
        for d in range(2):
            bg = b[l, d].reshape(4, 400)[GORDER]          # [g, j]
            # cols (jh_o, g, u): bg[g, jh_o*100+u]
            bb = bg.reshape(4, 4, 100).transpose(1, 0, 2).reshape(1600)
            bias_blk[l * 2 + d, 0] = bb
    out["bias_blk"] = bias_blk

    # ---- MLP weights [2 (l/r), 8 kt, 128 v, 100] ----
    wmlp = np.zeros((2, 8, 128, 100), f32)
    for i, W in enumerate((mlp_l_W, mlp_r_W)):
        for kt in range(8):
            din, jlh = kt // 4, kt % 4
            u_in = 32 * jlh + jll_v
            valid = u_in < 100
            xc = din * 400 + jh_v * 100 + np.minimum(u_in, 99)
            wmlp[i, kt] = (W[:, xc].T * valid[:, None]).astype(f32)
    out["wmlp"] = wmlp

    mlp_bias = np.zeros((2, 128, 1), f32)
    mlp_bias[0, :100, 0] = mlp_l_b
    mlp_bias[1, :100, 0] = mlp_r_b
    out["mlp_bias"] = mlp_bias

    # ---- biaffine [101 i, 128*101 (o,j)] ----
    out["biaf"] = np.ascontiguousarray(
        biaffine_W.transpose(1, 0, 2).reshape(101, NL * 101), f32)

    # matmul operands are bf16 on device
    import ml_dtypes
    bf16 = ml_dtypes.bfloat16
    for k in ("wm_all", "wih0", "wih12", "bias_blk", "wmlp", "biaf"):
        out[k] = out[k].astype(bf16)
    return out


def prep_core_tokens(words, tags, core):
    """Per-core token indices in (t, b) order, int32 [512, 1]."""
    bs = slice(4 * core, 4 * core + 4)
    w = np.ascontiguousarray(words[bs].T.reshape(NTOK, 1)).astype(np.int32)
    t = np.ascontiguousarray(tags[bs].T.reshape(NTOK, 1)).astype(np.int32)
    return w, t


# ================================ device build ==============================

def build(n_layers=N_LAYERS, do_head=DO_HEAD, debug_seq=DEBUG_SEQ):
    from concourse import bass, bacc, mybir, tile
    from concourse.masks import make_identity
    F32 = mybir.dt.float32
    BF16 = mybir.dt.bfloat16
    I32 = mybir.dt.int32
    AF = mybir.ActivationFunctionType

    nc = bacc.Bacc(None, target_bir_lowering=False)

    def mmr(out, lhsT, rhs, **kw):
        # bf16 matmul: 1 cycle/row (vs 4 for fp32); fp32 accumulate in PSUM
        nc.tensor.matmul(out=out, lhsT=lhsT, rhs=rhs, **kw)

    # ---- I/O ----
    words_tb = nc.dram_tensor("words_tb", [NTOK, 1], I32, kind="ExternalInput")
    tags_tb = nc.dram_tensor("tags_tb", [NTOK, 1], I32, kind="ExternalInput")
    word_emb = nc.dram_tensor("word_emb", [V, NE], F32, kind="ExternalInput")
    tag_emb = nc.dram_tensor("tag_emb", [NT, NE], F32, kind="ExternalInput")
    wm_all = nc.dram_tensor("wm_all", [6, 128, 6400], BF16, kind="ExternalInput")
    wih0_d = nc.dram_tensor("wih0", [2, 2, 128, 1600], BF16, kind="ExternalInput")
    wih12_d = nc.dram_tensor("wih12", [2, 2, 8, 128, 1600], BF16, kind="ExternalInput")
    bias_blk = nc.dram_tensor("bias_blk", [6, 128, 1600], BF16, kind="ExternalInput")
    wmlp_d = nc.dram_tensor("wmlp", [2, 8, 128, 100], BF16, kind="ExternalInput")
    mlp_bias_d = nc.dram_tensor("mlp_bias", [2, 128, 1], F32, kind="ExternalInput")
    biaf_d = nc.dram_tensor("biaf", [101, NL * 101], BF16, kind="ExternalInput")

    if do_head:
        # [b, y, x, o] layout (host swaps x/y back); bf16 halves output DMA
        out_d = nc.dram_tensor("out", [Bc, X, X, NL], BF16, kind="ExternalOutput")
    if debug_seq:
        dbg_seq = nc.dram_tensor("dbg_seq", [n_layers, 2, 128, T * 16], F32,
                                 kind="ExternalOutput")

    with tile.TileContext(nc) as tc:
        const = tc.alloc_tile_pool(name="const", bufs=1)
        dr = tc.alloc_tile_pool(name="dr", bufs=1, space="DRAM")
        seqp = tc.alloc_tile_pool(name="seqp", bufs=1)
        sb = tc.alloc_tile_pool(name="sb", bufs=1)
        sbr = tc.alloc_tile_pool(name="sbr", bufs=4)     # rotating small tiles
        ps = tc.alloc_tile_pool(name="ps", bufs=1, space="PSUM")
        psr = tc.alloc_tile_pool(name="psr", bufs=2, space="PSUM")

        # ---------- constants ----------
        id128 = const.tile([128, 128], F32, tag="id128", name="id128")
        make_identity(nc, id128[:])
        idr = const.tile([128, 128], BF16, tag="idr", name="idr")
        make_identity(nc, idr[:])
        ones_st = const.tile([128, 128], BF16, tag="ones_st", name="ones_st")   # row 0 = 1
        nc.vector.memset(ones_st[:], 0.0)
        nc.vector.memset(ones_st[0:1, :], 1.0)
        alpha_t = const.tile([128, 1], F32, tag="alpha", name="alpha")
        nc.vector.memset(alpha_t[:], 0.1)

        # ---------- DRAM scratch ----------
        proj_dram = [[dr.tile([NTOK, 1600], BF16, tag=f"proj{l}{d}", name=f"proj{l}{d}")
                      for d in range(2)] for l in range(n_layers)]

        # ---------- hidden sequences stay in SBUF (4KB/partition each) ----------
        # col layout (q, t, b) so per-(q) t-slices are contiguous: proj lhsT
        # and head rhs slices need a single flat free dim
        seq_slots = [[seqp.tile([128, T * 16], BF16, tag=f"seq{s}{d}", name=f"seq{s}{d}")
                      for d in range(2)] for s in range(2)]
        seq_sb = [seq_slots[l % 2] for l in range(n_layers)]

        # ---------- head weights: preload now, consumed after the layers ----
        wtiles = [[seqp.tile([128, 100], BF16, tag=f"wmlp{mi}{i}", name=f"wmlp{mi}{i}")
                   for i in range(8)] for mi in range(2)]
        bt_t = [seqp.tile([128, 1], F32, tag=f"mlpb{mi}", name=f"mlpb{mi}")
                for mi in range(2)]
        biaf_t = seqp.tile([101, NL * 101], BF16, tag="biaf_t", name="biaf_t")
        for mi in range(2):
            for kt in range(8):
                nc.gpsimd.dma_start(out=wtiles[mi][kt][:], in_=wmlp_d[mi, kt, :, :])
            nc.gpsimd.dma_start(out=bt_t[mi][:], in_=mlp_bias_d[mi, :, :])
        nc.gpsimd.dma_start(out=biaf_t[:], in_=biaf_d[:, :])
        head_w = (wtiles, bt_t, biaf_t)

        # ---------- embeddings -> x0T (feature-major) ----------
        x0T = [sb.tile([128, NTOK], BF16, tag=f"x0T{k}", name=f"x0T{k}") for k in range(2)]
        for k in range(2):
            nc.vector.memset(x0T[k][:], 0.0)
        gats = []
        for pt in range(4):
            tok_sl = slice(128 * pt, 128 * pt + 128)
            for k, (table, idx_d, nrow) in enumerate(
                    ((word_emb, words_tb, V), (tag_emb, tags_tb, NT))):
                idx_t = sbr.tile([128, 1], I32, tag="gidx", name="gidx", bufs=8)
                nc.sync.dma_start(out=idx_t[:], in_=idx_d[tok_sl, :])
                gat = sbr.tile([128, NE], F32, tag=f"gat{pt}{k}", name=f"gat{pt}{k}")
                nc.gpsimd.indirect_dma_start(
                    out=gat[:], out_offset=None, in_=table[:, :],
                    in_offset=bass.IndirectOffsetOnAxis(ap=idx_t[:, 0:1], axis=0))
                gats.append((gat, k, tok_sl))
        for gat, k, tok_sl in gats:
            tp = psr.tile([NE, 128], F32, tag="embT", name="embT", bufs=2)
            nc.tensor.transpose(out=tp[:], in_=gat[:], identity=id128[:])
            nc.scalar.copy(out=x0T[k][0:NE, tok_sl], in_=tp[:])

        # ---------- LSTM weights tiles (reloaded per layer) ----------
        wm_t = [sb.tile([128, 6400], BF16, tag=f"wm{d}", name=f"wm{d}") for d in range(2)]
        bias_t = [sb.tile([128, 1600], BF16, tag=f"bias{d}", name=f"bias{d}") for d in range(2)]
        # kt=3 weight tiles, parity-rotated: rows (v&31)<4 hold real weights,
        # rows 4..7 receive proj(t) by per-step DMA (dead weight rows hijacked;
        # matching indicator lives in h cols 100:104, see below)
        wm3_t = [[sb.tile([128, 1600], BF16, tag=f"wm3{d}{p}", name=f"wm3{d}{p}")
                  for p in range(2)] for d in range(2)]

        # persistent state tiles per dir
        c_t = [sb.tile([128, 100], F32, tag=f"c{d}", name=f"c{d}") for d in range(2)]
        h_t = [sb.tile([128, 128], BF16, tag=f"h{d}", name=f"h{d}") for d in range(2)]
        hT_t = [sb.tile([128, 128], BF16, tag=f"hT{d}", name=f"hT{d}") for d in range(2)]
        sg_t = [sb.tile([128, 300], F32, tag=f"sg{d}", name=f"sg{d}") for d in range(2)]
        tg_t = [sb.tile([128, 100], F32, tag=f"tg{d}", name=f"tg{d}") for d in range(2)]
        t1_t = [sb.tile([128, 100], F32, tag=f"t1{d}", name=f"t1{d}") for d in range(2)]
        t2_t = [sb.tile([128, 100], F32, tag=f"t2{d}", name=f"t2{d}") for d in range(2)]
        tc_t = [sb.tile([128, 100], F32, tag=f"tc{d}", name=f"tc{d}") for d in range(2)]
        gates = [ps.tile([128, 400], F32, tag=f"gates{d}", name=f"gates{d}") for d in range(2)]
        for d in range(2):
            nc.vector.memset(gates[d][:], 0.0)
            nc.vector.memset(h_t[d][:], 0.0)
            # proj indicator: hT rows 4..7 cols 96:100 become I4 via the
            # block transpose of h[0:4, 100:104] (cols >= 104 stay zero)
            nc.vector.tensor_copy(out=h_t[d][0:4, 100:104], in_=idr[0:4, 0:4])


        wih_t = [[sb.tile([128, 1600], BF16, tag=f"wih{d}{i}", name=f"wih{d}{i}")
                  for i in range(8)] for d in range(2)]
        for d in range(2):
            for kt in range(2):
                nc.gpsimd.dma_start(out=wih_t[d][kt][:], in_=wih0_d[d, kt, :, :])

        for l in range(n_layers):
            nkt = 2 if l == 0 else 8
            # -- load layer weights --
            for d in range(2):
                nc.gpsimd.dma_start(out=wm_t[d][:], in_=wm_all[l * 2 + d, :, :])
                nc.gpsimd.dma_start(out=bias_t[d][:], in_=bias_blk[l * 2 + d, :, :])
                for p in range(2):
                    nc.gpsimd.dma_start(out=wm3_t[d][p][:],
                                        in_=wm_all[l * 2 + d, :, 4800:6400])

            # -- x stationary APs for proj (straight out of SBUF seq) --
            if l > 0:
                sq = [seq_sb[l - 1][din][:].rearrange(
                    "p (q t b) -> p q t b", q=4, t=T, b=4) for din in range(2)]

            # -- proj big matmul, both dirs (weights prefetched) --
            for d in range(2):
                for pt in range(4):
                    tok_sl = slice(128 * pt, 128 * pt + 128)
                    pj_sb = sbr.tile([128, 1600], BF16, tag="pj_sb", name="pj_sb")
                    for jhp in range(2):
                        ppt = ps.tile([128, 1024], F32, tag="projps",
                                      name="projps", bufs=2)
                        pp2 = ppt[:].rearrange("p (a c) -> p a c", a=2, c=512)
                        for jj in range(2):
                            jh = 2 * jhp + jj
                            # bias first (start), then x contributions
                            mmr(out=pp2[:, jj, 0:400], lhsT=ones_st[:],
                                rhs=bias_t[d][:, jh * 400:(jh + 1) * 400],
                                start=True, stop=False, tile_position=(0, 0))
                            for kt in range(nkt):
                                if l == 0:
                                    lhsT = x0T[kt][:, tok_sl]
                                else:
                                    din, jlh = kt // 4, kt % 4
                                    lhsT = sq[din][:, jlh, 32 * pt:32 * pt + 32, :]
                                mmr(out=pp2[:, jj, 0:400], lhsT=lhsT,
                                    rhs=wih_t[d][kt][:, jh * 400:(jh + 1) * 400],
                                    start=False, stop=(kt == nkt - 1),
                                    tile_position=(0, 0))
                        eng = nc.scalar if jhp == 0 else nc.vector
                        dst = pj_sb[:].rearrange("p (a c) -> p a c", a=4, c=400)
                        if jhp == 0:
                            nc.scalar.copy(out=dst[:, 0:2, :], in_=pp2[:, :, 0:400])
                        else:
                            nc.vector.tensor_copy(out=dst[:, 2:4, :], in_=pp2[:, :, 0:400])
                    nc.sync.dma_start(out=proj_dram[l][d][tok_sl, :], in_=pj_sb[:])

            # prefetch next layer's proj weights; loads overlap the recurrence
            if l + 1 < n_layers:
                for d in range(2):
                    for kt in range(8):
                        nc.gpsimd.dma_start(out=wih_t[d][kt][:],
                                            in_=wih12_d[l, d, kt, :, :])

            # -- recurrence --
            for d in range(2):
                nc.vector.memset(c_t[d][:], 0.0)
                nc.vector.memset(hT_t[d][:], 0.0)
                # step-0 proj indicator in hT rows 4..8 (later steps get it
                # from the transpose of h[0:4, 100:104]); DMA because engine
                # writes cannot start at partition 4
                nc.sync.dma_start(out=hT_t[d][4:8, 96:100], in_=idr[0:4, 0:4])
                # prefetch proj for step 0 into wm3 parity tile rows 4..8
                tb0 = 0 if d == 0 else T - 1
                nc.sync.dma_start(out=wm3_t[d][tb0 % 2][4:8, :],
                                  in_=proj_dram[l][d][4 * tb0:4 * tb0 + 4, :])

            wm4s = [wm_t[d][:].rearrange("p (k a c) -> p k a c", k=4, a=4, c=400)
                    for d in range(2)]
            wm3v = [[wm3_t[d][p][:].rearrange("p (a c) -> p a c", a=4, c=400)
                     for p in range(2)] for d in range(2)]
            for t in range(T):
                tbs = (t, T - 1 - t)
                nxt = (t + 1, T - 2 - t)
                # matmuls (both dirs); proj rides in via wm3 rows 4..8
                for d in range(2):
                    tb = tbs[d]
                    g = gates[d]
                    if t + 1 < T:
                        nb = nxt[d]
                        nc.sync.dma_start(out=wm3_t[d][nb % 2][4:8, :],
                                          in_=proj_dram[l][d][4 * nb:4 * nb + 4, :])
                    for jh in range(4):
                        for kt in range(3):
                            mmr(out=g[32 * jh:32 * jh + 4, :],
                                lhsT=hT_t[d][:, 32 * kt:32 * kt + 4],
                                rhs=wm4s[d][:, kt, jh, :],
                                start=(kt == 0), stop=False,
                                tile_position=(0, 32 * jh))
                        mmr(out=g[32 * jh:32 * jh + 4, :],
                            lhsT=hT_t[d][:, 96:100],
                            rhs=wm3v[d][tb % 2][:, jh, :],
                            start=False, stop=True,
                            tile_position=(0, 32 * jh))
                # gate nonlinearities (ACT stream: sig_f, tanh_f, sig_b, tanh_b)
                for d in range(2):
                    nc.scalar.activation(out=sg_t[d][:], in_=gates[d][:, 0:300], func=AF.Sigmoid)
                    nc.scalar.activation(out=tg_t[d][:], in_=gates[d][:, 300:400], func=AF.Tanh)
                # cell state updates (DVE) + tanh(c) (ACT, queued after both sigmoids)
                for d in range(2):
                    nc.vector.tensor_mul(out=t1_t[d][:], in0=c_t[d][:], in1=sg_t[d][:, 100:200])
                    nc.vector.tensor_mul(out=t2_t[d][:], in0=sg_t[d][:, 0:100], in1=tg_t[d][:])
                    nc.vector.tensor_add(out=c_t[d][:], in0=t1_t[d][:], in1=t2_t[d][:])
                    nc.scalar.activation(out=tc_t[d][:], in_=c_t[d][:], func=AF.Tanh)
                # h, transpose, seq store (Pool copy into SBUF seq)
                for d in range(2):
                    tb = tbs[d]
                    nc.vector.tensor_mul(out=h_t[d][:, 0:100], in0=sg_t[d][:, 200:300], in1=tc_t[d][:])
                    nc.vector.transpose(out=hT_t[d][:], in_=h_t[d][:])
                    hT4 = hT_t[d][:].rearrange("p (a c) -> p a c", a=4, c=32)
                    nc.gpsimd.tensor_copy(
                        out=seq_sb[l][d][:].rearrange(
                            "p (q t b) -> p q t b", q=4, t=T, b=4)[:, :, tb, :],
                        in_=hT4[:, :, 0:4])

        if debug_seq:
            for l in range(n_layers):
                for d in range(2):
                    nc.sync.dma_start(out=dbg_seq[l, d, :, :], in_=seq_sb[l][d][:])

        for p in (psr, ps, sbr, sb):
            p.release()

        if do_head:
            _emit_head(nc, tc, seq_sb, head_w, alpha_t, out_d, mybir, mmr)
        seqp.release()

        for p in (dr, const):
            p.release()

    nc.finalize()
    return nc


def _emit_head(nc, tc, seq_dram, head_w, alpha_t, out_d, mybir, mmr):
    """Fencepost -> MLPs -> biaffine -> output.

    Biaffine runs in float32r (1 cyc/row needs out free >= 256):
      step 1: x padded to 128 -> N = 128*2 = 256 per o; o grouped x4 in PSUM
      step 2: x-chunks of 4 with full o -> N = 512
    Output written as [b, y, x, o] in bf16; host swaps x/y and upcasts.
    """
    F32 = mybir.dt.float32
    BF16 = mybir.dt.bfloat16
    AF = mybir.ActivationFunctionType
    NX = X * Bc  # 508
    XP = 128     # x padded (col 127 is zero garbage, never stored)
    sb = tc.alloc_tile_pool(name="hsb", bufs=1)
    sbr = tc.alloc_tile_pool(name="hsbr", bufs=4)
    psr = tc.alloc_tile_pool(name="hpsr", bufs=2, space="PSUM")

    sq = [seq_dram[-1][din][:].rearrange("p (q t b) -> p q t b", q=4, t=T, b=4)
          for din in range(2)]
    # fencepost x slices [128, 508] per kt=(din, jlh): contiguous in SBUF seq
    # (fwd t in [0,127), bwd t in [1,128)) -- used directly as matmul rhs
    xf_tiles = []
    for kt in range(8):
        din, jlh = kt // 4, kt % 4
        t0 = 0 if din == 0 else 1
        xf_tiles.append(sq[din][:, jlh, t0:t0 + X, :])

    # MLPs -> l1T / r1T [128 (101 used), 512]; weights preloaded at start
    wtiles, bt_t, biaf_t = head_w
    lr1T = []
    for mi in range(2):
        bt = bt_t[mi]
        mp = psr.tile([100, NX], F32, tag="mlp_ps", name="mlp_ps")
        for kt in range(8):
            mmr(out=mp[:], lhsT=wtiles[mi][kt][:, :], rhs=xf_tiles[kt],
                start=(kt == 0), stop=(kt == 7), tile_position=(0, 0))
        ot = sb.tile([128, 512], BF16, tag=f"lr1T{mi}", name=f"lr1T{mi}")
        nc.vector.memset(ot[:], 0.0)
        nc.vector.memset(ot[96:128, :], 1.0)  # row 100 -> 1.0; 96-99 overwritten below
        nc.scalar.activation(out=ot[0:100, 0:NX], in_=mp[:], func=AF.Prelu,
                             bias=bt[0:100, 0:1], alpha=alpha_t[0:100, 0:1])
        lr1T.append(ot)
    l1T, r1T = lr1T

    # biaffine weights were preloaded
    bsb = tc.alloc_tile_pool(name="bsb", bufs=1)
    biaf3 = biaf_t[:].rearrange("p (o j) -> p o j", o=NL, j=101)

    # tmp for ALL 4 batches: [101 j, (x_pad, 4, o)] bf16 = 128KB/partition
    tmp2 = bsb.tile([101, XP * 4 * NL], BF16, tag="tmp2", name="tmp2")
    tmp4 = tmp2[:].rearrange("p (x c o) -> p x c o", x=XP, c=4, o=NL)

    # x padded to 128: col 127 is zeros (l1T memset), harmless
    l1v = l1T[0:101, 0:512]                              # [101, 512] -> N=512
    r1v = r1T[0:101, 0:NX].rearrange("p (x c) -> p x c", x=X, c=4)
    XC = 4   # x-chunk for step 2 -> N = 512
    OG = 2   # o's per PSUM group in step 1 (1 bank each, 2 banks per group)

    # step 1: tmp[j, x, b, o] = sum_i W[o,i,j] * l1[b,x,i]  (all 4 batches)
    for og in range(0, NL, OG):
        ps1 = psr.tile([101, OG * 512], F32, tag="ps1", name="ps1")
        p1v = ps1[:].rearrange("p (o n) -> p o n", o=OG, n=512)
        for oi in range(OG):
            mmr(out=p1v[:, oi, :], lhsT=biaf3[:, og + oi, :], rhs=l1v,
                start=True, stop=True, tile_position=(0, 0))
        src = ps1[:].rearrange("p (o x c) -> p x c o", o=OG, x=XP, c=4)
        if (og // OG) % 2 == 0:
            nc.scalar.copy(out=tmp4[:, :, :, og:og + OG], in_=src)
        else:
            nc.vector.tensor_copy(out=tmp4[:, :, :, og:og + OG], in_=src)

    # step 2: s[b, x, y, o] = sum_j r1[b, y, j] * tmp[j, x, b, o]
    # Group 8 x-chunks (32 x's) per output DMA -> 8KB descriptors.
    # SWDGE (gpsimd) spreads packets over all 16 DMA engines but pays
    # ~50ns/descriptor generation on Pool; the HW qSP/qACT lane is one
    # ring (22.5 GB/s) with free generation -- give it ~1/8 of DMAs.
    XG = 2 * XC  # 8 x's per group: 2KB descriptors, many DMAs in flight
    dma_i = 0
    for b in range(4):
        r1_b = r1v[:, :, b]
        for x0 in range(0, X, XG):
            nx_tot = min(XG, X - x0)
            so = sbr.tile([X, XG * NL], BF16, tag="so", name="so", bufs=12)
            for sub in range(0, XG, XC):
                xh = x0 + sub
                if xh >= X:
                    continue
                nxh = min(XC, X - xh)
                ps2 = psr.tile([X, XC * NL], F32, tag="ps2", name="ps2")
                mmr(out=ps2[:], lhsT=r1_b, rhs=tmp4[:, xh:xh + XC, b, :],
                    start=True, stop=True, tile_position=(0, 0))
                csl = slice(sub * NL, sub * NL + nxh * NL)
                psl = slice(0, nxh * NL)
                if (sub // XC) % 2 == 0:
                    nc.scalar.copy(out=so[:, csl], in_=ps2[:, psl])
                else:
                    nc.vector.tensor_copy(out=so[:, csl], in_=ps2[:, psl])
            eng = (nc.sync if dma_i % 16 == 3 else
                   nc.scalar if dma_i % 16 == 11 else nc.gpsimd)
            dma_i += 1
            eng.dma_start(
                out=out_d[b, :, x0:x0 + nx_tot, :],
                in_=so[:, 0:nx_tot * NL].rearrange("p (x o) -> p x o",
                                                   x=nx_tot, o=NL))

    for p in (bsb, psr, sbr, sb):
        p.release()


# ================================ entry point ===============================

def run(np_in, trace=False, tmpdir=None):
    from concourse.bass_utils import run_bass_kernel_spmd

    shared = prep_shared(
        np_in["word_emb"], np_in["tag_emb"], np_in["Wih0"], np_in["Wih1"],
        np_in["Wih2"], np_in["Whh"], np_in["b"], np_in["mlp_l_W"],
        np_in["mlp_l_b"], np_in["mlp_r_W"], np_in["mlp_r_b"],
        np_in["biaffine_W"])

    nc = build()
    in_maps = []
    for c in range(NCORES):
        w, t = prep_core_tokens(np_in["words"], np_in["tags"], c)
        m = dict(shared)
        m["words_tb"] = w
        m["tags_tb"] = t
        in_maps.append(m)

    res = run_bass_kernel_spmd(nc, in_maps, core_ids=list(range(NCORES)),
                               trace=trace, tmpdir=tmpdir)
    out = np.empty((B, X, X, NL), np.float32)
    for c in range(NCORES):
        # device output is [b, y, x, o] bf16 -> upcast + swap to [b, x, y, o]
        oc = np.asarray(res.results[c]["out"]).astype(np.float32)
        out[4 * c:4 * c + 4] = oc.swapaxes(1, 2)
    return out, res


def kernel(**inputs):
    np_in = {k: np.asarray(v) for k, v in inputs.items()}
    out, _ = run(np_in)
    return out

